# revision 2
# baseline (speedup 1.0000x reference)
"""AttnBlock on 8 trn2 cores, fp8 v8 (no tensor_mask_reduce).

v4 -> v5: host-fused q2 projection (W2 = Wk^T Wq, single projection, no q);
Gram stats in two PSUM banks for overlapped diagonal extraction; S1 group
sums via one reduce + 32-wide masked max; small DMAs moved off the queue
head; hq on DVE+Pool (ACT holds only Sqrt+Exp, tables preloaded); ih=1
projection interleaved into stage C of ih=0; epilogue in 256-wide chunks
split across DVE/Pool.
"""

import numpy as np
import ml_dtypes

import concourse.bacc as bacc
import concourse.mybir as mybir
import concourse.tile as tile
from concourse.bass_utils import run_bass_kernel_spmd

P = 128
C = 512
N = 4096
NQ = 1024
KB = C // P
KP = KB // 2
JT = N // P
NPAIR = JT // 2
IH = 2
NT = N // 512
PREF = 4
EPS = 1e-6
SCALE = float(C) ** -0.5
SQ2 = 16.0
KEXP = 3.0
GRP = 65536.0      # elements per group (16 ch * 4096 tokens)

F32 = mybir.dt.float32
BF16 = mybir.dt.bfloat16
F8 = mybir.dt.float8e4
AF = mybir.ActivationFunctionType
ALU = mybir.AluOpType
DR = mybir.MatmulPerfMode.DoubleRow


def build_nc():
    nc = bacc.Bacc()

    x8 = nc.dram_tensor("x8", [C, N], F8, kind="ExternalInput")
    xt8 = nc.dram_tensor("xt8", [N, C], F8, kind="ExternalInput")
    xq = nc.dram_tensor("xq", [C, NQ], F32, kind="ExternalInput")
    xq8 = nc.dram_tensor("xq8", [C, NQ], F8, kind="ExternalInput")
    w2 = nc.dram_tensor("w2", [C, C], BF16, kind="ExternalInput")   # (WkT Wq).T
    wm = nc.dram_tensor("wm", [C, C], BF16, kind="ExternalInput")   # (Wo Wv).T
    gcol = nc.dram_tensor("gcol", [P, KB], F32, kind="ExternalInput")
    bcol = nc.dram_tensor("bcol", [P, KB], F32, kind="ExternalInput")
    bqc = nc.dram_tensor("bqc", [P, KB], F32, kind="ExternalInput")  # Wk^T bq
    boc = nc.dram_tensor("boc", [P, KB], F32, kind="ExternalInput")  # Wo bv + bo
    gavg = nc.dram_tensor("gavg", [P, P], F32, kind="ExternalInput")
    out = nc.dram_tensor("out", [C, NQ], F32, kind="ExternalOutput")

    x8_r = x8[:].rearrange("(blk p) n -> p blk n", p=P)
    xt8_r = xt8[:].rearrange("(jt p) c -> p jt c", p=P)
    xq_r = xq[:].rearrange("(blk p) n -> p blk n", p=P)
    xq8_r = xq8[:].rearrange("(blk p) n -> p blk n", p=P)
    out_r = out[:].rearrange("(blk p) n -> p blk n", p=P)
    w2_r = w2[:].rearrange("(kb p) co -> p kb co", p=P)
    wm_r = wm[:].rearrange("(kb p) co -> p kb co", p=P)

    with tile.TileContext(nc) as tc:
        with (
            tc.tile_pool(name="big", bufs=1) as big,
            tc.tile_pool(name="st", bufs=1) as st,
            tc.tile_pool(name="et", bufs=8) as etp,
            tc.tile_pool(name="ep", bufs=2) as ep,
            tc.tile_pool(name="epo", bufs=4) as epo,
            tc.tile_pool(name="ss", bufs=3, space="PSUM") as ssp,
            tc.tile_pool(name="pv", bufs=1, space="PSUM") as pvp,
        ):
            x8_sb = big.tile([P, KB, N], F8)
            xt8_sb = big.tile([P, JT, C], F8)
            xq_sb = big.tile([P, KB, NQ], F32)
            xq8_sb = big.tile([P, KB, NQ], F8)
            w2_sb = big.tile([P, KB, C], BF16)
            wm_sb = big.tile([P, KB, C], BF16)
            gcol_sb = big.tile([P, KB], F32)
            bcol_sb = big.tile([P, KB], F32)
            bqc_sb = big.tile([P, KB], F32)
            boc_sb = big.tile([P, KB], F32)
            gavg_sb = big.tile([P, P], F32)
            ones1 = big.tile([P, 2, 16], F8)
            kbias = big.tile([P, 1], F32)
            eps_sb = big.tile([P, 1], F32)
            hq_bf = big.tile([P, KB, NQ], BF16)
            q28 = big.tile([P, KB, NQ], F8)
            gsel = big.tile([P, KB, P], F32)
            s1b = big.tile([P, 512], F32)
            expdump = big.tile([P, 1], BF16)

            nc.vector.memset(ones1, 1.0)
            nc.vector.memset(kbias, -KEXP)
            nc.vector.memset(eps_sb, EPS)
            # preload the Exp table while ACT is idle
            nc.scalar.activation(out=expdump, in_=eps_sb, func=AF.Exp)

            # ---- DMA stream ----
            s1ps = pvp.tile([P, 512], F32, tag="pv0", name="s1ps")
            s1psB = pvp.tile([P, 512], F32, tag="pv3", name="s1psB")
            s1rowA = st.tile([1, 512], F32)
            gps = [pvp.tile([P, 512], F32, tag="pv1", name="gp0"),
                   pvp.tile([P, 512], F32, tag="pv2", name="gp1"),
                   ssp.tile([P, 512], F32, tag="ss", name="gp2"),
                   ssp.tile([P, 512], F32, tag="ss", name="gp3")]
            gsbA = big.tile([P, KB, P], F32)
            gsbB = big.tile([P, KB, P], F32)
            s2a = st.tile([P, KB], F32)
            s2b = st.tile([P, KB], F32)

            def gram_half(prs, first, last):
                for pr in prs:
                    sdst = s1ps if pr < 8 else s1psB
                    nc.tensor.matmul(
                        sdst[0:1, :], ones1[:, :, 0:1],
                        xt8_sb[:, 2 * pr:2 * pr + 2, :],
                        start=(pr == first), stop=(pr == last), perf_mode=DR)
                    for cc in range(KB):
                        sl = xt8_sb[:, 2 * pr:2 * pr + 2, cc * P:(cc + 1) * P]
                        nc.tensor.matmul(
                            gps[cc][:, 0:P], sl, sl,
                            start=(pr == first), stop=(pr == last),
                            perf_mode=DR)

            for nt in range(NT):
                nc.sync.dma_start(out=xt8_sb[:, 4 * nt:4 * nt + 4, :],
                                  in_=xt8_r[:, 4 * nt:4 * nt + 4, :])
                if nt < 4:
                    gram_half((2 * nt, 2 * nt + 1), 0, 7)
                else:
                    gram_half((2 * nt, 2 * nt + 1), 8, 15)
                if nt == 2:
                    nc.sync.dma_start(out=gavg_sb, in_=gavg[:])
                if nt == 3:
                    # A-half complete -> evacuate and extract early
                    for cc in range(KB):
                        nc.vector.tensor_copy(out=gsbA[:, cc, :],
                                              in_=gps[cc][:, 0:P])
                    nc.gpsimd.affine_select(
                        out=gsel[:, :, :], in_=gsbA,
                        pattern=[[0, KB], [1, P]], compare_op=ALU.is_equal,
                        fill=0.0, base=0, channel_multiplier=-1)
                    nc.vector.reduce_sum(out=s2a, in_=gsel,
                                         axis=mybir.AxisListType.X)
                    nc.vector.tensor_copy(out=s1rowA, in_=s1ps[0:1, :])
            nc.sync.dma_start(out=xq8_sb, in_=xq8_r)
            nc.sync.dma_start(out=w2_sb, in_=w2_r)
            nc.sync.dma_start(out=gcol_sb, in_=gcol[:])
            nc.sync.dma_start(out=bcol_sb, in_=bcol[:])
            nc.sync.dma_start(out=bqc_sb, in_=bqc[:])
            nc.sync.dma_start(out=boc_sb, in_=boc[:])
            for nt in range(NT):
                off = nt * 512
                nc.sync.dma_start(out=x8_sb[:, :, off:off + 512],
                                  in_=x8_r[:, :, off:off + 512])
            nc.sync.dma_start(out=wm_sb, in_=wm_r)
            nc.sync.dma_start(out=xq_sb, in_=xq_r)

            # ---- stats extraction (affine_select diagonals) ----
            gselB = st.tile([P, KB, P], F32)
            for cc in range(KB):
                nc.vector.tensor_copy(out=gsbB[:, cc, :], in_=gps[cc][:, 0:P])
            nc.gpsimd.affine_select(
                out=gselB[:, :, :], in_=gsbB,
                pattern=[[0, KB], [1, P]], compare_op=ALU.is_equal,
                fill=0.0, base=0, channel_multiplier=-1)
            nc.vector.reduce_sum(out=s2b, in_=gselB, axis=mybir.AxisListType.X)
            s1rowB = st.tile([1, 512], F32)
            nc.vector.tensor_copy(out=s1rowB, in_=s1psB[0:1, :])
            nc.vector.tensor_tensor(out=s1rowB, in0=s1rowB, in1=s1rowA,
                                    op=ALU.add)
            nc.gpsimd.partition_broadcast(s1b, s1rowB)
            gsel1 = st.tile([P, KB, P], F32)
            nc.gpsimd.affine_select(
                out=gsel1[:, :, :],
                in_=s1b[:, :].rearrange("p (a b) -> p a b", a=KB),
                pattern=[[0, KB], [1, P]], compare_op=ALU.is_equal,
                fill=0.0, base=0, channel_multiplier=-1)
            s1col = st.tile([P, KB], F32)
            nc.vector.reduce_sum(out=s1col, in_=gsel1, axis=mybir.AxisListType.X)
            stat8 = st.tile([P, 8], F32)
            nc.vector.tensor_copy(out=stat8[:, 0:4], in_=s1col)
            nc.vector.tensor_tensor(out=stat8[:, 4:8], in0=s2a, in1=s2b,
                                    op=ALU.add)
            psb = pvp.tile([P, 512], F32, tag="pv3", name="psb")
            nc.tensor.matmul(psb[:, 0:8], gavg_sb, stat8, start=True, stop=True)
            mucol = st.tile([P, KB], F32)
            nc.vector.tensor_copy(out=mucol, in_=psb[:, 0:4])
            varg = st.tile([P, 4], F32)
            nc.vector.tensor_tensor(out=varg, in0=mucol, in1=mucol, op=ALU.mult)
            nc.vector.tensor_tensor(out=varg, in0=psb[:, 4:8], in1=varg,
                                    op=ALU.subtract)
            rstd = st.tile([P, 4], F32)
            nc.scalar.activation(out=rstd, in_=varg, func=AF.Sqrt, bias=eps_sb)
            nc.vector.reciprocal(out=rstd, in_=rstd)
            Acol = st.tile([P, 4], F32)
            Dcol = st.tile([P, 4], F32)
            nc.vector.tensor_tensor(out=Acol, in0=rstd, in1=gcol_sb, op=ALU.mult)
            nc.vector.tensor_tensor(out=Dcol, in0=mucol, in1=Acol, op=ALU.mult)
            nc.vector.tensor_tensor(out=Dcol, in0=bcol_sb, in1=Dcol, op=ALU.subtract)
            sq2c = st.tile([P, 4], F32)
            nc.vector.tensor_scalar_mul(sq2c, Acol, SCALE * SQ2)
            D_bf = st.tile([P, 4], BF16)
            nc.vector.tensor_copy(out=D_bf, in_=Dcol)

            # ---- bias folds: qb2 = W2 D + Wk^T bq ; obias = M D + Wo bv + bo ----
            qbraw = st.tile([P, KB], F32)
            mdraw = st.tile([P, KB], F32)
            for ob in range(KB):
                qbp = pvp.tile([P, 512], F32, tag=f"pv{ob}", name=f"qbp{ob}")
                for kb in range(KB):
                    nc.tensor.matmul(qbp[:, 0:1],
                                     w2_sb[:, kb, ob * P:(ob + 1) * P],
                                     D_bf[:, kb:kb + 1],
                                     start=(kb == 0), stop=(kb == KB - 1))
                nc.vector.tensor_copy(out=qbraw[:, ob:ob + 1], in_=qbp[:, 0:1])
            qb2c = st.tile([P, KB], F32)
            nc.vector.tensor_tensor(out=qb2c, in0=qbraw, in1=bqc_sb, op=ALU.add)
            for ob in range(KB):
                mdp = pvp.tile([P, 512], F32, tag=f"pv{ob}", name=f"mdp{ob}")
                for kb in range(KB):
                    nc.tensor.matmul(mdp[:, 0:1],
                                     wm_sb[:, kb, ob * P:(ob + 1) * P],
                                     D_bf[:, kb:kb + 1],
                                     start=(kb == 0), stop=(kb == KB - 1))
                nc.vector.tensor_copy(out=mdraw[:, ob:ob + 1], in_=mdp[:, 0:1])
            obias_col = st.tile([P, KB], F32)
            nc.vector.tensor_tensor(out=obias_col, in0=mdraw, in1=boc_sb,
                                    op=ALU.add)

            # PE warmup bridge
            wmt = ssp.tile([P, 512], F32, tag="ss", name="wmt")
            for i in range(10):
                nc.tensor.matmul(wmt[0:16, :], ones1[:, :, 0:16],
                                 xt8_sb[:, 0:2, 0:512],
                                 start=(i == 0), stop=(i == 9), perf_mode=DR)

            # ---- stage B: hq (DVE+Pool), W2 projection for ih=0 ----
            for kb in range(KB):
                eng = nc.gpsimd if kb == 2 else nc.vector
                eng.tensor_scalar_mul(
                    hq_bf[:, kb, :], xq8_sb[:, kb, :], Acol[:, kb:kb + 1])

            def proj_ih(ih):
                pst = {}
                for blk in range(KB):
                    pst[blk] = pvp.tile([P, 512], F32, tag=f"pv{blk}",
                                        name=f"k{ih}{blk}")
                for kb in range(KB):
                    for blk in range(KB):
                        nc.tensor.matmul(
                            pst[blk], w2_sb[:, kb, blk * P:(blk + 1) * P],
                            hq_bf[:, kb, ih * 512:(ih + 1) * 512],
                            start=(kb == 0), stop=(kb == KB - 1))
                for blk in range(KB):
                    nc.vector.tensor_scalar(
                        out=q28[:, blk, ih * 512:(ih + 1) * 512],
                        in0=pst[blk], scalar1=qb2c[:, blk:blk + 1],
                        scalar2=sq2c[:, blk:blk + 1], op0=ALU.add, op1=ALU.mult)

            proj_ih(0)

            # ---- stage C ----
            pv_t = {}
            sd_t = {}

            def emit_scores(ih, m):
                et = etp.tile([P, 2, 512], F8, tag="et", name=f"et{ih}_{m}")
                for t in range(2):
                    jt = 2 * m + t
                    ss = ssp.tile([P, 512], F32, tag="ss", name=f"ss{ih}_{jt}")
                    for kp in range(KP):
                        nc.tensor.matmul(
                            ss, x8_sb[:, 2 * kp:2 * kp + 2, jt * P:(jt + 1) * P],
                            q28[:, 2 * kp:2 * kp + 2, ih * 512:(ih + 1) * 512],
                            start=(kp == 0), stop=(kp == KP - 1), perf_mode=DR)
                    nc.scalar.activation(out=et[:, t, :], in_=ss, func=AF.Exp,
                                         bias=kbias, scale=1.0 / SQ2)
                return et

            def emit_pv(ih, mm, et):
                nc.tensor.matmul(
                    sd_t[ih][0:1, :], ones1[:, :, 0:1], et[:, :, :],
                    start=(mm == 0), stop=(mm == NPAIR - 1), perf_mode=DR)
                for cc in range(KB):
                    nc.tensor.matmul(
                        pv_t[ih][cc],
                        xt8_sb[:, 2 * mm:2 * mm + 2, cc * P:(cc + 1) * P],
                        et[:, :, :], start=(mm == 0), stop=(mm == NPAIR - 1),
                        perf_mode=DR)

            def epilogue(ih):
                rec = ep.tile([1, 512], F32, tag="rec", name=f"rec{ih}")
                nc.vector.reciprocal(out=rec, in_=sd_t[ih][0:1, :])
                rbs = ep.tile([P, 512], F32, tag="rbs", name=f"rbs{ih}")
                nc.gpsimd.partition_broadcast(rbs, rec)
                on_bf = ep.tile([P, KB, 512], BF16, tag="on", name=f"on{ih}")
                po = {}
                for blk in range(KB):
                    po[blk] = pvp.tile([P, 512], F32, tag=f"pv{blk}",
                                       name=f"po{ih}{blk}")
                for ch in range(2):
                    cs = slice(ch * 256, (ch + 1) * 256)
                    for cc in range(KB):
                        if cc < 2:
                            nc.vector.tensor_scalar_mul(
                                on_bf[:, cc, cs], pv_t[ih][cc][:, cs],
                                Acol[:, cc:cc + 1])
                        else:
                            nc.scalar.activation(
                                out=on_bf[:, cc, cs], in_=pv_t[ih][cc][:, cs],
                                func=AF.Copy, scale=Acol[:, cc:cc + 1])
                    for blk in range(KB):
                        for cc in range(KB):
                            nc.tensor.matmul(
                                po[blk][:, cs],
                                wm_sb[:, cc, blk * P:(blk + 1) * P],
                                on_bf[:, cc, cs],
                                start=(cc == 0), stop=(cc == KB - 1))
                        t1 = epo.tile([P, 256], F32, tag="t1",
                                      name=f"t1{ih}{blk}{ch}")
                        ot = epo.tile([P, 256], F32, tag="ot",
                                      name=f"ot{ih}{blk}{ch}")
                        e2 = nc.vector if blk < 2 else nc.gpsimd
                        nc.vector.tensor_tensor(out=t1, in0=po[blk][:, cs],
                                                in1=rbs[:, cs], op=ALU.mult)
                        e2.tensor_tensor(
                            out=ot, in0=t1,
                            in1=xq_sb[:, blk, ih * 512 + ch * 256:
                                      ih * 512 + ch * 256 + 256], op=ALU.add)
                        nc.sync.dma_start(
                            out=out_r[:, blk, ih * 512 + ch * 256:
                                      ih * 512 + ch * 256 + 256], in_=ot)

            # obias fold into residual on idle DVE
            for blk in range(KB):
                nc.vector.tensor_scalar_add(
                    xq_sb[:, blk, :], xq_sb[:, blk, :], obias_col[:, blk:blk + 1])

            # ---- ih = 0 (with ih=1 projection interleaved) ----
            sd_t[0] = pvp.tile([P, 512], F32, tag="sd", name="sd0")
            pv_t[0] = [pvp.tile([P, 512], F32, tag=f"pv{cc}", name=f"pv0_{cc}")
                       for cc in range(KB)]
            ets0 = {}
            for m in range(NPAIR):
                ets0[m] = emit_scores(0, m)
                if m in (2, 4, 6, 8):
                    blk = m // 2 - 1
                    kps = ssp.tile([P, 512], F32, tag="ss", name=f"k1{blk}")
                    for kb in range(KB):
                        nc.tensor.matmul(
                            kps, w2_sb[:, kb, blk * P:(blk + 1) * P],
                            hq_bf[:, kb, 512:1024],
                            start=(kb == 0), stop=(kb == KB - 1))
                    nc.vector.tensor_scalar(
                        out=q28[:, blk, 512:1024], in0=kps,
                        scalar1=qb2c[:, blk:blk + 1],
                        scalar2=sq2c[:, blk:blk + 1], op0=ALU.add, op1=ALU.mult)
                if m >= 1:
                    emit_pv(0, m - 1, ets0.pop(m - 1))
            emit_pv(0, NPAIR - 1, ets0.pop(NPAIR - 1))

            # ---- ih = 1 preface, epilogue(0), rest ----
            sd_t[1] = pvp.tile([P, 512], F32, tag="sd", name="sd1")
            ets1 = {}
            for m in range(PREF):
                ets1[m] = emit_scores(1, m)
            epilogue(0)
            pv_t[1] = [pvp.tile([P, 512], F32, tag=f"pv{cc}", name=f"pv1_{cc}")
                       for cc in range(KB)]
            nxt = 0
            for m in range(PREF, NPAIR + 1):
                if m < NPAIR:
                    ets1[m] = emit_scores(1, m)
                avail = (m + 1 if m < NPAIR else NPAIR)
                want = 2 if m >= 8 else 1
                while nxt < avail - 1 and want > 0:
                    emit_pv(1, nxt, ets1.pop(nxt))
                    nxt += 1
                    want -= 1
            while nxt < NPAIR:
                emit_pv(1, nxt, ets1.pop(nxt))
                nxt += 1
            epilogue(1)

    nc.finalize()
    return nc


_NC = None


def _get_nc():
    global _NC
    if _NC is None:
        _NC = build_nc()
    return _NC


def _col(v):
    return np.ascontiguousarray(np.asarray(v, np.float32).reshape(KB, P).T)


def _make_in_maps(inputs):
    F8NP = ml_dtypes.float8_e4m3
    BF = ml_dtypes.bfloat16
    x = np.asarray(inputs["x"], np.float32).reshape(2, C, N)
    Wq = np.asarray(inputs["Wq"], np.float32)
    Wk = np.asarray(inputs["Wk"], np.float32)
    Wv = np.asarray(inputs["Wv"], np.float32)
    Wo = np.asarray(inputs["Wo"], np.float32)
    W2 = Wk.T @ Wq                      # q2 = W2 @ hn + Wk^T bq
    M = Wo @ Wv
    wk2bq = Wk.T @ np.asarray(inputs["bq"], np.float32)
    wobv = Wo @ np.asarray(inputs["bv"], np.float32) + np.asarray(
        inputs["bo"], np.float32)
    pidx = np.arange(P)
    gavg = np.where(pidx[:, None] // 16 == pidx[None, :] // 16,
                    np.float32(1.0 / GRP), np.float32(0.0)).astype(np.float32)
    common = dict(
        w2=np.ascontiguousarray(W2.T).astype(BF),
        wm=np.ascontiguousarray(M.T).astype(BF),
        gcol=_col(inputs["gamma"]), bcol=_col(inputs["beta"]),
        bqc=_col(wk2bq), boc=_col(wobv), gavg=gavg)
    in_maps = []
    for core in range(8):
        b, qc = core // 4, core % 4
        xb8 = np.ascontiguousarray(x[b]).astype(F8NP)
        in_maps.append(dict(
            common,
            x8=xb8,
            xt8=np.ascontiguousarray(x[b].T).astype(F8NP),
            xq=np.ascontiguousarray(x[b][:, qc * NQ:(qc + 1) * NQ]),
            xq8=np.ascontiguousarray(xb8[:, qc * NQ:(qc + 1) * NQ]),
        ))
    return in_maps


def run(inputs, trace=False):
    nc = _get_nc()
    in_maps = _make_in_maps(inputs)
    res = run_bass_kernel_spmd(nc, in_maps, core_ids=list(range(8)), trace=trace)
    y = np.empty((2, C, N), np.float32)
    for core in range(8):
        b, qc = core // 4, core % 4
        y[b][:, qc * NQ:(qc + 1) * NQ] = res.results[core]["out"]
    return y.reshape(2, C, 64, 64), res


def kernel(**inputs):
    y, _ = run(inputs, trace=False)
    return y
